# revision 2
# baseline (speedup 1.0000x reference)
"""Trainium2 Bass kernel for nn_AttnPlus, v4 (S^T orientation).

Per core (8 cores = 4 batches x 2 head-groups, 8 local heads):
  LN -> qk projection (fp8 DoubleRow) -> per head: transposed scores
  S^T = k q^T (fp8 DoubleRow, contraction 64 packed as 32x2), exp on
  ScalarE (-> fp8) or VectorE (Schraudolph bit-exp -> int16 viewed as
  bf16), then softmax numerator+denominator via PE reduction matmuls
  with lhsT=[v*wv*VS | VS] accumulated over key chunks in PSUM.
  Heads of a quad land at PSUM partitions {0,32,64,96} (col tiling).
  Final att = num/den assembled via small PE transposes.

Self-contained: hardcodes shapes from the problem spec.
"""

import numpy as np
import ml_dtypes

B, N, DIM, HEAD = 4, 2048, 1024, 16
HD = DIM // HEAD            # 64
HPC = HEAD // 2             # heads per core = 8
NCORES = 8
EPS = 1e-5
P = 128
NT = N // P                 # 16 row tiles
DC = DIM // P               # 8 contraction chunks
MC = 16                     # m (key) chunks of 128
MCP = 8                     # m chunk pairs
NQH = 2                     # q halves of 1024
QW = N // NQH               # 1024

SC = float(DIM ** -0.5) / 256.0   # exp scale (qk carry x16 each)
# Schraudolph bit-exp emitting fp8e4m3 bit patterns via int8 write:
# bits(2^y) ~ 8*(y+7) + minimax correction
BA8 = SC * 8.0 / float(np.log(2.0))
BB8 = 56.0 - 8.0 * 0.0430

# chunk-engine assignment: unit u -> ACT exp if ASSIGN_PATTERN[u % 8]
# (else DVE bit-exp). 5/8 ACT matches the engine balance model.
ASSIGN_PATTERN = (1, 0, 1, 0, 1, 1, 0, 1, 1, 0, 1, 0, 1, 0, 1, 0)
ND_LAG = 2       # units of score-matmul lookahead before each nd matmul

_CACHE = {}


def _legalize_bir(raw: bytes) -> bytes:
    """This container's walrus allows only one sync-wait command per
    instruction; Tile emits several. Split extras onto same-engine NoOp
    carriers inserted immediately before."""
    import orjson

    m = orjson.loads(raw)
    for fn in m.get("functions", []):
        for b in fn.get("basic_blocks", fn.get("blocks", [])):
            insts = b.get("instructions", [])
            out = []
            changed = False
            for i in insts:
                si = i.get("sync_info")
                waits = si.get("on_wait") if si else None
                if waits and len(waits) > 1:
                    changed = True
                    for k, w in enumerate(waits[:-1]):
                        out.append({
                            "name": f"{i['name']}-sw{k}",
                            "opcode": "NoOp",
                            "engine": i["engine"],
                            "ins": [],
                            "outs": [],
                            "debug": i.get("debug", 0),
                            "sync_info": {"on_wait": [w], "on_update": []},
                        })
                    si["on_wait"] = [waits[-1]]
                out.append(i)
            if changed:
                b["instructions"] = out
    return orjson.dumps(m)


def _build_bass():
    import concourse.bass as bass
    import concourse.tile as tile
    from concourse import mybir
    from concourse.masks import make_identity
    from contextlib import ExitStack

    f32 = mybir.dt.float32
    bf16 = mybir.dt.bfloat16
    fp8 = mybir.dt.float8e4
    i16 = mybir.dt.int16
    Alu = mybir.AluOpType
    Act = mybir.ActivationFunctionType
    DR = mybir.MatmulPerfMode.DoubleRow

    nc = bass.Bass()
    x_d = nc.dram_tensor("x", [N, DIM], f32, kind="ExternalInput")
    wt_d = nc.dram_tensor("wt", [P, DC // 2, 2, DIM], fp8, kind="ExternalInput")
    bias_d = nc.dram_tensor("bias", [DIM], f32, kind="ExternalInput")
    a_d = nc.dram_tensor("a", [N, HPC], f32, kind="ExternalInput")
    vdr8_d = nc.dram_tensor("vdr8", [P, HPC, 2, MCP, 2], fp8, kind="ExternalInput")
    out_d = nc.dram_tensor("out", [N, HPC], f32, kind="ExternalOutput")

    with tile.TileContext(nc) as tc, ExitStack() as ctx:
        persist = ctx.enter_context(tc.tile_pool(name="persist", bufs=1))
        xp = ctx.enter_context(tc.tile_pool(name="xp", bufs=3))
        xnp = ctx.enter_context(tc.tile_pool(name="xnp", bufs=4))
        stat = ctx.enter_context(tc.tile_pool(name="stat", bufs=6))
        e8p = ctx.enter_context(tc.tile_pool(name="e8p", bufs=8))
        e16p = ctx.enter_context(tc.tile_pool(name="e16p", bufs=8))
        ndsbp = ctx.enter_context(tc.tile_pool(name="ndsbp", bufs=8))
        attp = ctx.enter_context(tc.tile_pool(name="attp", bufs=4))
        sp = ctx.enter_context(tc.tile_pool(name="sp", bufs=3, space="PSUM"))
        ndp = ctx.enter_context(tc.tile_pool(name="ndp", bufs=1, space="PSUM"))

        # ---------- persistent tensors ----------
        wT = persist.tile([P, DC // 2, 2, DIM], fp8, tag="wT", name="wT")
        xnT = persist.tile([P, DC, N], fp8, tag="xnT", name="xnT")
        kT = persist.tile([P, 4, N], fp8, tag="kT", name="kT")
        qA = persist.tile([P, 4, N], fp8, tag="qA", name="qA")
        qB = persist.tile([P, 4, N], fp8, tag="qB", name="qB")
        vdr8_sb = persist.tile([P, HPC, 2, MCP, 2], fp8, tag="v8", name="v8")
        bias_sb = persist.tile([P, DC], f32, tag="bias_sb", name="bias_sb")
        id_bf = persist.tile([P, P], bf16, tag="id_bf", name="id_bf")
        id_f32 = persist.tile([P, P], f32, tag="id_f32", name="id_f32")
        eps_sb = persist.tile([P, 1], f32, tag="eps_sb", name="eps_sb")
        # output rows q = qh*1024 + c8*128 + nw; partition = c8
        o_sb = persist.tile([8, NQH, P, HPC], f32, tag="o_sb", name="o_sb")
        a2 = persist.tile([8, NQH, P, HPC], f32, tag="a2", name="a2")

        # ---------- constants + input DMAs ----------
        make_identity(nc, id_bf)
        make_identity(nc, id_f32)
        nc.vector.memset(eps_sb, EPS)
        nc.sync.dma_start(out=bias_sb, in_=bias_d.rearrange("(ec p) -> p ec", p=P))
        nc.sync.dma_start(
            out=a2,
            in_=a_d.rearrange("(qh c8 nw) h -> c8 qh nw h", qh=NQH, c8=8),
        )
        nc.sync.dma_start(out=wT, in_=wt_d.ap())
        nc.gpsimd.dma_start(out=vdr8_sb, in_=vdr8_d.ap())
        out_v = out_d.rearrange("(qh c8 nw) h -> c8 qh nw h", qh=NQH, c8=8)
        nc.gpsimd.memset(qA[64:128, :, :], 0.0)
        nc.gpsimd.memset(qB[0:64, :, :], 0.0)
        x_r = x_d.rearrange("(t p) d -> t p d", p=P)

        # ---------- projection: psum = W' @ xn^T, fp8 out with bias ----------
        # group g covers packed-e columns g*128..+128; one jt slice at a time
        # so it interleaves with LayerNorm (needs only LN tiles 4jt..4jt+3)
        def proj_g_jt(g, jt):
            pr = g // 2
            pj = sp.tile([P, 512], f32, tag="ps", name="pj")
            for dd2 in range(DC // 2):
                nc.tensor.matmul(
                    out=pj,
                    lhsT=wT[:, dd2, :, g * P : (g + 1) * P],
                    rhs=xnT[:, 2 * dd2 : 2 * dd2 + 2, jt * 512 : (jt + 1) * 512],
                    perf_mode=DR,
                    start=(dd2 == 0), stop=(dd2 == DC // 2 - 1),
                )
            sl = slice(jt * 512, (jt + 1) * 512)
            if g % 2 == 0:
                nc.scalar.activation(
                    out=qA[0:64, pr, sl], in_=pj[0:64, :], func=Act.Identity,
                    bias=bias_sb[0:64, g : g + 1], scale=1.0,
                )
                nc.vector.tensor_scalar(
                    out=qB[64:128, pr, sl], in0=pj[64:128, :],
                    scalar1=bias_sb[64:128, g : g + 1], scalar2=None,
                    op0=Alu.add,
                )
            else:
                nc.scalar.activation(
                    out=kT[:, pr, sl], in_=pj, func=Act.Identity,
                    bias=bias_sb[:, g : g + 1], scale=1.0,
                )

        # ---------- LayerNorm + transpose into xnT ----------
        for t in range(NT):
            xt = xp.tile([P, DIM], f32, tag="xt", name="xt")
            nc.sync.dma_start(out=xt, in_=x_r[t])
            st = stat.tile([P, 2, 6], f32, tag="st", name="st")
            nc.vector.bn_stats(out=st[:, 0, :], in_=xt[:, 0:512])
            nc.vector.bn_stats(out=st[:, 1, :], in_=xt[:, 512:1024])
            mv = stat.tile([P, 2], f32, tag="mv", name="mv")
            nc.vector.bn_aggr(out=mv, in_=st)
            rstd = stat.tile([P, 1], f32, tag="rstd", name="rstd")
            nc.scalar.activation(out=rstd, in_=mv[:, 1:2], func=Act.Sqrt, bias=eps_sb)
            nc.vector.reciprocal(out=rstd, in_=rstd)
            nmr = stat.tile([P, 1], f32, tag="nmr", name="nmr")
            nc.vector.scalar_tensor_tensor(
                out=nmr, in0=mv[:, 0:1], scalar=-1.0, in1=rstd,
                op0=Alu.mult, op1=Alu.mult,
            )
            xn_t = xnp.tile([P, DIM], bf16, tag="xn", name="xn")
            if t % 2 == 0:
                nc.scalar.activation(
                    out=xn_t, in_=xt, func=Act.Identity, bias=nmr, scale=rstd,
                )
            else:
                nc.vector.tensor_scalar(
                    out=xn_t, in0=xt, scalar1=mv[:, 0:1], scalar2=rstd,
                    op0=Alu.subtract, op1=Alu.mult,
                )
            tp = sp.tile([P, DIM], bf16, tag="ps", name="tp")
            for dd in range(DC):
                nc.tensor.transpose(
                    out=tp[:, dd * P : (dd + 1) * P],
                    in_=xn_t[:, dd * P : (dd + 1) * P],
                    identity=id_bf,
                )
            if t % 2 == 0:
                nc.vector.tensor_copy(
                    out=xnT[:, :, t * P : (t + 1) * P],
                    in_=tp.rearrange("p (dc n) -> p dc n", dc=DC),
                )
            else:
                nc.scalar.copy(
                    out=xnT[:, :, t * P : (t + 1) * P],
                    in_=tp.rearrange("p (dc n) -> p dc n", dc=DC),
                )

        # ---------- softmax quad: 4 heads, S^T orientation ----------
        proj_queue = []
        ucnt = [0]       # unit counter for engine assignment
        step = [0]       # global unit counter for deferred emission
        deferred = []    # (due_step, emit_fn): drains/att pushed into the
                         # following stretch so they never head-of-line-block
                         # the consumer engine queues

        seq = [0]

        def tick():
            step[0] += 1
            due = sorted([i for i in deferred if i[0] <= step[0]],
                         key=lambda x: (x[0], x[1]))
            for item in due:
                deferred.remove(item)
                item[2]()
            if proj_queue and step[0] % 2 == 0:
                g, jt = proj_queue.pop(0)
                proj_g_jt(g, jt)

        def drain_all():
            for item in sorted(deferred, key=lambda x: (x[0], x[1])):
                deferred.remove(item)
                item[2]()

        def quad(qd):
            for qh in range(NQH):
                ndsbs = [None] * 4
                for hh in range(4):
                    pr = qd * 2 + hh // 2
                    r = hh % 2            # head within pair: qA or qB
                    h = qd * 4 + hh
                    qsrc = qA if r == 0 else qB
                    nd = ndp.tile([2, QW], f32, tag="nd", name="nd")
                    pending = []

                    def flush_nd(keep, nd=nd, h=h, pending=pending):
                        while len(pending) > keep:
                            mcp_, rhs8_ = pending.pop(0)
                            for jt in range(2):
                                nc.tensor.matmul(
                                    out=nd[0:2, jt * 512 : (jt + 1) * 512],
                                    lhsT=vdr8_sb[:, h, :, mcp_, :],
                                    rhs=rhs8_[:, :, jt * 512 : (jt + 1) * 512],
                                    perf_mode=DR,
                                    start=(mcp_ == 0), stop=(mcp_ == MCP - 1),
                                    skip_group_check=True,
                                )

                    for mcp in range(MCP):
                        is_act = bool(ASSIGN_PATTERN[ucnt[0] % len(ASSIGN_PATTERN)])
                        ucnt[0] += 1
                        if is_act:
                            et = e8p.tile([P, 2, QW], fp8, tag="e8", name="e8")
                            rhs8 = et
                        else:
                            et = e16p.tile([P, 2, QW], mybir.dt.int8, tag="ei", name="ei")
                            rhs8 = et[:, :, :].bitcast(fp8)
                        for j in range(2):
                            mc = 2 * mcp + j
                            s_ps = sp.tile([P, QW], f32, tag="ps", name="s_ps")
                            for jt in range(2):
                                nc.tensor.matmul(
                                    out=s_ps[:, jt * 512 : (jt + 1) * 512],
                                    lhsT=kT[:, pr, mc * P : (mc + 1) * P],
                                    rhs=qsrc[:, pr,
                                             qh * QW + jt * 512 : qh * QW + (jt + 1) * 512],
                                    start=True, stop=True,
                                )
                            if is_act:
                                nc.scalar.activation(
                                    out=et[:, j, :], in_=s_ps, func=Act.Exp, scale=SC,
                                )
                            else:
                                nc.vector.tensor_scalar(
                                    out=et[:, j, :], in0=s_ps, scalar1=BA8,
                                    scalar2=BB8, op0=Alu.mult, op1=Alu.add,
                                )
                        pending.append((mcp, rhs8))
                        flush_nd(2)
                        tick()

                    def emit_drain(nd=nd, hh=hh, flush_nd=flush_nd,
                                   ndsbs=ndsbs):
                        flush_nd(0)
                        ndsb = ndsbp.tile([2, QW], f32, tag="ndsb", name="ndsb")
                        if hh % 2 == 0:
                            nc.vector.tensor_copy(out=ndsb, in_=nd)
                        else:
                            nc.scalar.copy(out=ndsb, in_=nd)
                        ndsbs[hh] = ndsb
                    seq[0] += 1
                    deferred.append((step[0] + 2, seq[0], emit_drain))

                def emit_att(qd=qd, qh=qh, ndsbs=ndsbs):
                    # transpose 128-q blocks: [2, 128] -> [128, 2] per head
                    atp = sp.tile([P, 64], f32, tag="ps", name="atp")
                    for c8 in range(8):
                        for hh in range(4):
                            nc.tensor.transpose(
                                out=atp[:, (c8 * 4 + hh) * 2 : (c8 * 4 + hh) * 2 + 2],
                                in_=ndsbs[hh][:, c8 * P : (c8 + 1) * P],
                                identity=id_f32[0:2, 0:2],
                            )
                    # atp cols = (c8, head, {num, den})
                    ap_v = atp.rearrange("p (c h k) -> p c h k", h=4, k=2)
                    rden = attp.tile([P, 32], f32, tag="rden", name="rden")
                    rd_v = rden.rearrange("p (c h) -> p c h", h=4)
                    nc.vector.reciprocal(
                        out=rden, in_=ap_v[:, :, :, 1].rearrange("p c h -> p (c h)"),
                    )
                    atv = attp.tile([P, P], f32, tag="atv", name="atv")
                    nc.vector.memset(atv, 0.0)
                    av = atv.rearrange("p (h c) -> p h c", h=4)
                    for head in range(4):
                        nc.vector.tensor_tensor(
                            out=av[:, head, 0:8],
                            in0=ap_v[:, :, head, 0],
                            in1=rd_v[:, :, head], op=Alu.mult,
                        )
                    o_ps = sp.tile([P, P], f32, tag="ps", name="o_ps")
                    nc.tensor.transpose(out=o_ps, in_=atv, identity=id_f32)
                    for head in range(4):
                        nc.vector.tensor_tensor(
                            out=o_sb[0:8, qh, :, qd * 4 + head],
                            in0=o_ps[head * 32 : head * 32 + 8, :],
                            in1=a2[0:8, qh, :, qd * 4 + head], op=Alu.add,
                        )
                    nc.sync.dma_start(
                        out=out_v[:, qh, :, qd * 4 : qd * 4 + 4],
                        in_=o_sb[0:8, qh, :, qd * 4 : qd * 4 + 4],
                    )
                seq[0] += 1
                deferred.append((step[0] + 3, seq[0], emit_att))

        for g in range(2):
            for jt in range(4):
                proj_g_jt(g, jt)
        proj_queue.extend((g, jt) for g in range(2, 8) for jt in range(4))
        quad(0)
        quad(1)
        drain_all()



    fixed = _legalize_bir(nc.to_json_bytes())
    nc.to_json_bytes = lambda: fixed
    return nc


def _host_prep(x, A, ln_w, ln_b, Wqk, wv):
    bf = ml_dtypes.bfloat16
    f8 = ml_dtypes.float8_e4m3
    W = (Wqk.astype(np.float32) * ln_w.astype(np.float32)[None, :])
    bias_full = Wqk.astype(np.float32) @ ln_b.astype(np.float32)
    W = W * 16.0
    bias_full = bias_full * 16.0

    wvf = float(np.asarray(wv).reshape(-1)[0])
    vall = A[..., 0].astype(np.float32) * wvf          # [B, N, HEAD]
    vmax = float(np.abs(vall).max()) + 1e-30
    VS = float(2.0 ** np.floor(np.log2(400.0 / vmax)))
    VS = float(np.clip(VS, 2.0 ** -8, 2.0 ** 8))

    in_maps = []
    meta = []
    for core in range(NCORES):
        b, g = core // 2, core % 2
        # packed-e order (pair packing): ec 2p = q of heads (2p, 2p+1),
        # ec 2p+1 = k of the same pair; head A in cols 0-63, B in 64-127
        heads = list(range(g * HPC, (g + 1) * HPC))
        e_order = []
        for p in range(HPC // 2):
            h0, h1 = heads[2 * p], heads[2 * p + 1]
            e_order += list(range(h0 * HD, (h0 + 1) * HD))
            e_order += list(range(h1 * HD, (h1 + 1) * HD))
            e_order += list(range(DIM + h0 * HD, DIM + (h0 + 1) * HD))
            e_order += list(range(DIM + h1 * HD, DIM + (h1 + 1) * HD))
        e_order = np.asarray(e_order)
        wt = np.ascontiguousarray(
            W[e_order].T.reshape(4, 2, 128, DIM).transpose(2, 0, 1, 3)
            .astype(f8))
        bias_c = np.ascontiguousarray(bias_full[e_order].astype(np.float32))

        v = vall[b, :, g * HPC : (g + 1) * HPC]         # [N, 8]
        vs = (v * VS).astype(np.float32)                # scaled values
        # vdr8 [P, HPC, 2(ko), MCP, 2(m)]
        vdr8 = np.zeros((P, HPC, 2, MCP, 2), dtype=np.float32)
        for h in range(HPC):
            for mc in range(MC):
                seg = vs[mc * P : (mc + 1) * P, h]
                vdr8[:, h, mc % 2, mc // 2, 0] = seg
                vdr8[:, h, mc % 2, mc // 2, 1] = VS
        in_maps.append({
            "x": np.ascontiguousarray(x[b].astype(np.float32)),
            "wt": wt,
            "bias": bias_c,
            "a": np.ascontiguousarray(
                A[b, :, g * HPC : (g + 1) * HPC, 0].astype(np.float32)),
            "vdr8": np.ascontiguousarray(vdr8.astype(f8)),
        })
        meta.append((b, g))
    return in_maps, meta


LAST_EXEC_NS = None


def kernel(x, A, ln_w, ln_b, Wqk, wv):
    global LAST_EXEC_NS
    import os
    from concourse.bass_utils import run_bass_kernel_spmd

    x = np.asarray(x); A = np.asarray(A)
    ln_w = np.asarray(ln_w); ln_b = np.asarray(ln_b)
    Wqk = np.asarray(Wqk); wv = np.asarray(wv)

    if "nc" not in _CACHE:
        _CACHE["nc"] = _build_bass()
    nc = _CACHE["nc"]

    in_maps, meta = _host_prep(x, A, ln_w, ln_b, Wqk, wv)
    trace = bool(int(os.environ.get("ATTN_TRACE", "0")))
    res = run_bass_kernel_spmd(
        nc, in_maps, core_ids=list(range(NCORES)), trace=trace,
    )
    LAST_EXEC_NS = res.exec_time_ns

    out = np.zeros((B, N, HEAD, 1), dtype=np.float32)
    for core, (b, g) in enumerate(meta):
        out[b, :, g * HPC : (g + 1) * HPC, 0] = res.results[core]["out"]
    return out


# revision 3
# speedup vs baseline: 1.0024x; 1.0024x over previous
"""Trainium2 Bass kernel for nn_AttnPlus (transposed-softmax design).

Per core (8 cores = 4 batches x 2 head-groups, 8 local heads):
  LayerNorm -> qk projection (fp8 DoubleRow matmuls) -> per head,
  TRANSPOSED scores S^T = k q^T as plain fp8 matmuls whose contraction
  is zero-padded to 128 rows (keeps the PE's HAM clock-gate released),
  exp split across ScalarE (table exp -> fp8e4m3) and VectorE
  (Schraudolph bit-exp -> int8 bit pattern viewed as fp8), and the
  softmax numerator+denominator computed on the PE as DoubleRow
  reduction matmuls with lhsT=[v*wv*VS | VS] accumulated over key
  chunks in PSUM -- removing all per-element softmax reduction work
  from the vector engines. nd drains and attention assembly are
  deferred into the following head's stretch so they never block the
  engine FIFOs; nd reduction matmuls trail the score stream by a few
  units for the same reason.

Self-contained: hardcodes shapes from the problem spec.
"""

import numpy as np
import ml_dtypes

B, N, DIM, HEAD = 4, 2048, 1024, 16
HD = DIM // HEAD            # 64
HPC = HEAD // 2             # heads per core = 8
NCORES = 8
EPS = 1e-5
P = 128
NT = N // P                 # 16 row tiles
DC = DIM // P               # 8 contraction chunks
MC = 16                     # m (key) chunks of 128
MCP = 8                     # m chunk pairs
NQH = 2                     # q halves of 1024
QW = N // NQH               # 1024

SC = float(DIM ** -0.5) / 256.0   # exp scale (qk carry x16 each)
# Schraudolph bit-exp emitting fp8e4m3 bit patterns via int8 write:
# bits(2^y) ~ 8*(y+7) + minimax correction
BA8 = SC * 8.0 / float(np.log(2.0))
BB8 = 56.0 - 8.0 * 0.0430

# chunk-engine assignment: unit u -> ACT exp if ASSIGN_PATTERN[u % 8]
# (else DVE bit-exp). 5/8 ACT matches the engine balance model.
ASSIGN_PATTERN = (1, 0, 1, 0, 1, 0, 1, 0, 1, 0, 1, 0, 1, 0, 1, 1)
ND_LAG = 2       # units of score-matmul lookahead before each nd matmul

_CACHE = {}


def _legalize_bir(raw: bytes) -> bytes:
    """This container's walrus allows only one sync-wait command per
    instruction; Tile emits several. Split extras onto same-engine NoOp
    carriers inserted immediately before."""
    import orjson

    m = orjson.loads(raw)
    for fn in m.get("functions", []):
        for b in fn.get("basic_blocks", fn.get("blocks", [])):
            insts = b.get("instructions", [])
            out = []
            changed = False
            for i in insts:
                si = i.get("sync_info")
                waits = si.get("on_wait") if si else None
                if waits and len(waits) > 1:
                    changed = True
                    for k, w in enumerate(waits[:-1]):
                        out.append({
                            "name": f"{i['name']}-sw{k}",
                            "opcode": "NoOp",
                            "engine": i["engine"],
                            "ins": [],
                            "outs": [],
                            "debug": i.get("debug", 0),
                            "sync_info": {"on_wait": [w], "on_update": []},
                        })
                    si["on_wait"] = [waits[-1]]
                out.append(i)
            if changed:
                b["instructions"] = out
    return orjson.dumps(m)


def _build_bass():
    import concourse.bass as bass
    import concourse.tile as tile
    from concourse import mybir
    from concourse.masks import make_identity
    from contextlib import ExitStack

    f32 = mybir.dt.float32
    bf16 = mybir.dt.bfloat16
    fp8 = mybir.dt.float8e4
    i16 = mybir.dt.int16
    Alu = mybir.AluOpType
    Act = mybir.ActivationFunctionType
    DR = mybir.MatmulPerfMode.DoubleRow

    nc = bass.Bass()
    x_d = nc.dram_tensor("x", [N, DIM], f32, kind="ExternalInput")
    wt_d = nc.dram_tensor("wt", [P, DC // 2, 2, DIM], fp8, kind="ExternalInput")
    bias_d = nc.dram_tensor("bias", [DIM], f32, kind="ExternalInput")
    a_d = nc.dram_tensor("a", [N, HPC], f32, kind="ExternalInput")
    vdr8_d = nc.dram_tensor("vdr8", [P, HPC, 2, MCP, 2], fp8, kind="ExternalInput")
    out_d = nc.dram_tensor("out", [N, HPC], f32, kind="ExternalOutput")

    with tile.TileContext(nc) as tc, ExitStack() as ctx:
        persist = ctx.enter_context(tc.tile_pool(name="persist", bufs=1))
        xp = ctx.enter_context(tc.tile_pool(name="xp", bufs=3))
        xnp = ctx.enter_context(tc.tile_pool(name="xnp", bufs=4))
        stat = ctx.enter_context(tc.tile_pool(name="stat", bufs=6))
        e8p = ctx.enter_context(tc.tile_pool(name="e8p", bufs=8))
        e16p = ctx.enter_context(tc.tile_pool(name="e16p", bufs=8))
        ndsbp = ctx.enter_context(tc.tile_pool(name="ndsbp", bufs=8))
        attp = ctx.enter_context(tc.tile_pool(name="attp", bufs=4))
        sp = ctx.enter_context(tc.tile_pool(name="sp", bufs=3, space="PSUM"))
        ndp = ctx.enter_context(tc.tile_pool(name="ndp", bufs=1, space="PSUM"))

        # ---------- persistent tensors ----------
        wT = persist.tile([P, DC // 2, 2, DIM], fp8, tag="wT", name="wT")
        xnT = persist.tile([P, DC, N], fp8, tag="xnT", name="xnT")
        kT = persist.tile([P, 4, N], fp8, tag="kT", name="kT")
        qA = persist.tile([P, 4, N], fp8, tag="qA", name="qA")
        qB = persist.tile([P, 4, N], fp8, tag="qB", name="qB")
        vdr8_sb = persist.tile([P, HPC, 2, MCP, 2], fp8, tag="v8", name="v8")
        bias_sb = persist.tile([P, DC], f32, tag="bias_sb", name="bias_sb")
        id_bf = persist.tile([P, P], bf16, tag="id_bf", name="id_bf")
        id_f32 = persist.tile([P, P], f32, tag="id_f32", name="id_f32")
        eps_sb = persist.tile([P, 1], f32, tag="eps_sb", name="eps_sb")
        # output rows q = qh*1024 + c8*128 + nw; partition = c8
        o_sb = persist.tile([8, NQH, P, HPC], f32, tag="o_sb", name="o_sb")
        a2 = persist.tile([8, NQH, P, HPC], f32, tag="a2", name="a2")

        # ---------- constants + input DMAs ----------
        make_identity(nc, id_bf)
        make_identity(nc, id_f32)
        nc.vector.memset(eps_sb, EPS)
        nc.sync.dma_start(out=bias_sb, in_=bias_d.rearrange("(ec p) -> p ec", p=P))
        nc.sync.dma_start(
            out=a2,
            in_=a_d.rearrange("(qh c8 nw) h -> c8 qh nw h", qh=NQH, c8=8),
        )
        nc.sync.dma_start(out=wT, in_=wt_d.ap())
        nc.gpsimd.dma_start(out=vdr8_sb, in_=vdr8_d.ap())
        out_v = out_d.rearrange("(qh c8 nw) h -> c8 qh nw h", qh=NQH, c8=8)
        nc.gpsimd.memset(qA[64:128, :, :], 0.0)
        nc.gpsimd.memset(qB[0:64, :, :], 0.0)
        x_r = x_d.rearrange("(t p) d -> t p d", p=P)

        # ---------- projection: psum = W' @ xn^T, fp8 out with bias ----------
        # group g covers packed-e columns g*128..+128; one jt slice at a time
        # so it interleaves with LayerNorm (needs only LN tiles 4jt..4jt+3)
        def proj_g_jt(g, jt):
            pr = g // 2
            pj = sp.tile([P, 512], f32, tag="ps", name="pj")
            for dd2 in range(DC // 2):
                nc.tensor.matmul(
                    out=pj,
                    lhsT=wT[:, dd2, :, g * P : (g + 1) * P],
                    rhs=xnT[:, 2 * dd2 : 2 * dd2 + 2, jt * 512 : (jt + 1) * 512],
                    perf_mode=DR,
                    start=(dd2 == 0), stop=(dd2 == DC // 2 - 1),
                )
            sl = slice(jt * 512, (jt + 1) * 512)
            if g % 2 == 0:
                nc.scalar.activation(
                    out=qA[0:64, pr, sl], in_=pj[0:64, :], func=Act.Identity,
                    bias=bias_sb[0:64, g : g + 1], scale=1.0,
                )
                nc.vector.tensor_scalar(
                    out=qB[64:128, pr, sl], in0=pj[64:128, :],
                    scalar1=bias_sb[64:128, g : g + 1], scalar2=None,
                    op0=Alu.add,
                )
            else:
                nc.scalar.activation(
                    out=kT[:, pr, sl], in_=pj, func=Act.Identity,
                    bias=bias_sb[:, g : g + 1], scale=1.0,
                )

        # ---------- LayerNorm + transpose into xnT ----------
        for t in range(NT):
            xt = xp.tile([P, DIM], f32, tag="xt", name="xt")
            nc.sync.dma_start(out=xt, in_=x_r[t])
            st = stat.tile([P, 2, 6], f32, tag="st", name="st")
            nc.vector.bn_stats(out=st[:, 0, :], in_=xt[:, 0:512])
            nc.vector.bn_stats(out=st[:, 1, :], in_=xt[:, 512:1024])
            mv = stat.tile([P, 2], f32, tag="mv", name="mv")
            nc.vector.bn_aggr(out=mv, in_=st)
            rstd = stat.tile([P, 1], f32, tag="rstd", name="rstd")
            nc.scalar.activation(out=rstd, in_=mv[:, 1:2], func=Act.Sqrt, bias=eps_sb)
            nc.vector.reciprocal(out=rstd, in_=rstd)
            nmr = stat.tile([P, 1], f32, tag="nmr", name="nmr")
            nc.vector.scalar_tensor_tensor(
                out=nmr, in0=mv[:, 0:1], scalar=-1.0, in1=rstd,
                op0=Alu.mult, op1=Alu.mult,
            )
            xn_t = xnp.tile([P, DIM], bf16, tag="xn", name="xn")
            if t % 2 == 0:
                nc.scalar.activation(
                    out=xn_t, in_=xt, func=Act.Identity, bias=nmr, scale=rstd,
                )
            else:
                nc.vector.tensor_scalar(
                    out=xn_t, in0=xt, scalar1=mv[:, 0:1], scalar2=rstd,
                    op0=Alu.subtract, op1=Alu.mult,
                )
            tp = sp.tile([P, DIM], bf16, tag="ps", name="tp")
            for dd in range(DC):
                nc.tensor.transpose(
                    out=tp[:, dd * P : (dd + 1) * P],
                    in_=xn_t[:, dd * P : (dd + 1) * P],
                    identity=id_bf,
                )
            if t % 2 == 0:
                nc.vector.tensor_copy(
                    out=xnT[:, :, t * P : (t + 1) * P],
                    in_=tp.rearrange("p (dc n) -> p dc n", dc=DC),
                )
            else:
                nc.scalar.copy(
                    out=xnT[:, :, t * P : (t + 1) * P],
                    in_=tp.rearrange("p (dc n) -> p dc n", dc=DC),
                )

        # ---------- softmax quad: 4 heads, S^T orientation ----------
        proj_queue = []
        ucnt = [0]       # unit counter for engine assignment
        step = [0]       # global unit counter for deferred emission
        deferred = []    # (due_step, emit_fn): drains/att pushed into the
                         # following stretch so they never head-of-line-block
                         # the consumer engine queues

        seq = [0]

        def tick():
            step[0] += 1
            due = sorted([i for i in deferred if i[0] <= step[0]],
                         key=lambda x: (x[0], x[1]))
            for item in due:
                deferred.remove(item)
                item[2]()
            if proj_queue and step[0] % 2 == 0:
                g, jt = proj_queue.pop(0)
                proj_g_jt(g, jt)

        def drain_all():
            for item in sorted(deferred, key=lambda x: (x[0], x[1])):
                deferred.remove(item)
                item[2]()

        def quad(qd):
            for qh in range(NQH):
                ndsbs = [None] * 4
                for hh in range(4):
                    pr = qd * 2 + hh // 2
                    r = hh % 2            # head within pair: qA or qB
                    h = qd * 4 + hh
                    qsrc = qA if r == 0 else qB
                    nd = ndp.tile([2, QW], f32, tag="nd", name="nd")
                    pending = []

                    def flush_nd(keep, nd=nd, h=h, pending=pending):
                        while len(pending) > keep:
                            mcp_, rhs8_ = pending.pop(0)
                            for jt in range(2):
                                nc.tensor.matmul(
                                    out=nd[0:2, jt * 512 : (jt + 1) * 512],
                                    lhsT=vdr8_sb[:, h, :, mcp_, :],
                                    rhs=rhs8_[:, :, jt * 512 : (jt + 1) * 512],
                                    perf_mode=DR,
                                    start=(mcp_ == 0), stop=(mcp_ == MCP - 1),
                                    skip_group_check=True,
                                )

                    for mcp in range(MCP):
                        is_act = bool(ASSIGN_PATTERN[ucnt[0] % len(ASSIGN_PATTERN)])
                        ucnt[0] += 1
                        if is_act:
                            et = e8p.tile([P, 2, QW], fp8, tag="e8", name="e8")
                            rhs8 = et
                        else:
                            et = e16p.tile([P, 2, QW], mybir.dt.int8, tag="ei", name="ei")
                            rhs8 = et[:, :, :].bitcast(fp8)
                        for j in range(2):
                            mc = 2 * mcp + j
                            s_ps = sp.tile([P, QW], f32, tag="ps", name="s_ps")
                            for jt in range(2):
                                nc.tensor.matmul(
                                    out=s_ps[:, jt * 512 : (jt + 1) * 512],
                                    lhsT=kT[:, pr, mc * P : (mc + 1) * P],
                                    rhs=qsrc[:, pr,
                                             qh * QW + jt * 512 : qh * QW + (jt + 1) * 512],
                                    start=True, stop=True,
                                )
                            if is_act:
                                nc.scalar.activation(
                                    out=et[:, j, :], in_=s_ps, func=Act.Exp, scale=SC,
                                )
                            else:
                                nc.vector.tensor_scalar(
                                    out=et[:, j, :], in0=s_ps, scalar1=BA8,
                                    scalar2=BB8, op0=Alu.mult, op1=Alu.add,
                                )
                        pending.append((mcp, rhs8))
                        flush_nd(3)
                        tick()

                    def emit_drain(nd=nd, hh=hh, flush_nd=flush_nd,
                                   ndsbs=ndsbs):
                        flush_nd(0)
                        ndsb = ndsbp.tile([2, QW], f32, tag="ndsb", name="ndsb")
                        if hh % 2 == 0:
                            nc.vector.tensor_copy(out=ndsb, in_=nd)
                        else:
                            nc.scalar.copy(out=ndsb, in_=nd)
                        ndsbs[hh] = ndsb
                    seq[0] += 1
                    deferred.append((step[0] + 2, seq[0], emit_drain))

                def emit_att(qd=qd, qh=qh, ndsbs=ndsbs):
                    # transpose 128-q blocks: [2, 128] -> [128, 2] per head
                    atp = sp.tile([P, 64], f32, tag="ps", name="atp")
                    for c8 in range(8):
                        for hh in range(4):
                            nc.tensor.transpose(
                                out=atp[:, (c8 * 4 + hh) * 2 : (c8 * 4 + hh) * 2 + 2],
                                in_=ndsbs[hh][:, c8 * P : (c8 + 1) * P],
                                identity=id_f32[0:2, 0:2],
                            )
                    # atp cols = (c8, head, {num, den})
                    ap_v = atp.rearrange("p (c h k) -> p c h k", h=4, k=2)
                    rden = attp.tile([P, 32], f32, tag="rden", name="rden")
                    rd_v = rden.rearrange("p (c h) -> p c h", h=4)
                    nc.vector.reciprocal(
                        out=rden, in_=ap_v[:, :, :, 1].rearrange("p c h -> p (c h)"),
                    )
                    atv = attp.tile([P, P], f32, tag="atv", name="atv")
                    nc.vector.memset(atv, 0.0)
                    av = atv.rearrange("p (h c) -> p h c", h=4)
                    for head in range(4):
                        nc.vector.tensor_tensor(
                            out=av[:, head, 0:8],
                            in0=ap_v[:, :, head, 0],
                            in1=rd_v[:, :, head], op=Alu.mult,
                        )
                    o_ps = sp.tile([P, P], f32, tag="ps", name="o_ps")
                    nc.tensor.transpose(out=o_ps, in_=atv, identity=id_f32)
                    for head in range(4):
                        nc.vector.tensor_tensor(
                            out=o_sb[0:8, qh, :, qd * 4 + head],
                            in0=o_ps[head * 32 : head * 32 + 8, :],
                            in1=a2[0:8, qh, :, qd * 4 + head], op=Alu.add,
                        )
                    nc.sync.dma_start(
                        out=out_v[:, qh, :, qd * 4 : qd * 4 + 4],
                        in_=o_sb[0:8, qh, :, qd * 4 : qd * 4 + 4],
                    )
                seq[0] += 1
                deferred.append((step[0] + 3, seq[0], emit_att))

        for g in range(2):
            for jt in range(4):
                proj_g_jt(g, jt)
        proj_queue.extend((g, jt) for g in range(2, 8) for jt in range(4))
        quad(0)
        quad(1)
        drain_all()



    fixed = _legalize_bir(nc.to_json_bytes())
    nc.to_json_bytes = lambda: fixed
    return nc


def _host_prep(x, A, ln_w, ln_b, Wqk, wv):
    bf = ml_dtypes.bfloat16
    f8 = ml_dtypes.float8_e4m3
    W = (Wqk.astype(np.float32) * ln_w.astype(np.float32)[None, :])
    bias_full = Wqk.astype(np.float32) @ ln_b.astype(np.float32)
    W = W * 16.0
    bias_full = bias_full * 16.0

    wvf = float(np.asarray(wv).reshape(-1)[0])
    vall = A[..., 0].astype(np.float32) * wvf          # [B, N, HEAD]
    vmax = float(np.abs(vall).max()) + 1e-30
    VS = float(2.0 ** np.floor(np.log2(400.0 / vmax)))
    VS = float(np.clip(VS, 2.0 ** -8, 2.0 ** 8))

    in_maps = []
    meta = []
    for core in range(NCORES):
        b, g = core // 2, core % 2
        # packed-e order (pair packing): ec 2p = q of heads (2p, 2p+1),
        # ec 2p+1 = k of the same pair; head A in cols 0-63, B in 64-127
        heads = list(range(g * HPC, (g + 1) * HPC))
        e_order = []
        for p in range(HPC // 2):
            h0, h1 = heads[2 * p], heads[2 * p + 1]
            e_order += list(range(h0 * HD, (h0 + 1) * HD))
            e_order += list(range(h1 * HD, (h1 + 1) * HD))
            e_order += list(range(DIM + h0 * HD, DIM + (h0 + 1) * HD))
            e_order += list(range(DIM + h1 * HD, DIM + (h1 + 1) * HD))
        e_order = np.asarray(e_order)
        wt = np.ascontiguousarray(
            W[e_order].T.reshape(4, 2, 128, DIM).transpose(2, 0, 1, 3)
            .astype(f8))
        bias_c = np.ascontiguousarray(bias_full[e_order].astype(np.float32))

        v = vall[b, :, g * HPC : (g + 1) * HPC]         # [N, 8]
        vs = (v * VS).astype(np.float32)                # scaled values
        # vdr8 [P, HPC, 2(ko), MCP, 2(m)]
        vdr8 = np.zeros((P, HPC, 2, MCP, 2), dtype=np.float32)
        for h in range(HPC):
            for mc in range(MC):
                seg = vs[mc * P : (mc + 1) * P, h]
                vdr8[:, h, mc % 2, mc // 2, 0] = seg
                vdr8[:, h, mc % 2, mc // 2, 1] = VS
        in_maps.append({
            "x": np.ascontiguousarray(x[b].astype(np.float32)),
            "wt": wt,
            "bias": bias_c,
            "a": np.ascontiguousarray(
                A[b, :, g * HPC : (g + 1) * HPC, 0].astype(np.float32)),
            "vdr8": np.ascontiguousarray(vdr8.astype(f8)),
        })
        meta.append((b, g))
    return in_maps, meta


LAST_EXEC_NS = None


def kernel(x, A, ln_w, ln_b, Wqk, wv):
    global LAST_EXEC_NS
    import os
    from concourse.bass_utils import run_bass_kernel_spmd

    x = np.asarray(x); A = np.asarray(A)
    ln_w = np.asarray(ln_w); ln_b = np.asarray(ln_b)
    Wqk = np.asarray(Wqk); wv = np.asarray(wv)

    if "nc" not in _CACHE:
        _CACHE["nc"] = _build_bass()
    nc = _CACHE["nc"]

    in_maps, meta = _host_prep(x, A, ln_w, ln_b, Wqk, wv)
    trace = bool(int(os.environ.get("ATTN_TRACE", "0")))
    res = run_bass_kernel_spmd(
        nc, in_maps, core_ids=list(range(NCORES)), trace=trace,
    )
    LAST_EXEC_NS = res.exec_time_ns

    out = np.zeros((B, N, HEAD, 1), dtype=np.float32)
    for core, (b, g) in enumerate(meta):
        out[b, :, g * HPC : (g + 1) * HPC, 0] = res.results[core]["out"]
    return out
